# revision 14
# baseline (speedup 1.0000x reference)
"""Trainium2 Bass kernel for ConvertedLlamaAttention (LoRA q/k/v + RoPE + causal attention + out-proj).

Strategy: tensor-parallel over heads across 8 NeuronCores (4 heads/core).
All device matmuls run in "transposed" layouts so no on-device transposes are
needed anywhere:
  - Q^T, K^T computed as W^T-stationary matmuls (head_dim on partitions),
  - V computed in natural layout (seq on partitions) from the same X^T tiles,
  - scores computed transposed (S^T = K^T^T-slices @ Q^T) so softmax sums are
    done with a ones-vector matmul, and A·V consumes V in natural layout,
  - out-proj consumes A·V^T directly as the stationary operand.
LoRA (incl. the half-interleave) is folded into the weights on the host.
Each core emits a partial (2048, 4096) bf16 output (row-parallel Wo); host sums.

v2 over baseline:
  - all DRAM layouts host-packed for wide contiguous DMA lines,
  - rope: scalar-engine PSUM evacuation to bf16, DVE muls in bf16 (4x mode),
  - bf16 everywhere off the accumulators (qt/kt/v/probs/avs/wot/out),
  - softmax-normalization tail software-pipelined one unit behind,
  - probs pair pre-add on DVE halves the ones-matmul count,
  - Wo resident in SBUF, bf16 output with 2KB lines halves write traffic.
"""
import sys

for _p in ("/opt/trn_rl_repo", "/root/.axon_site/_ro/trn_rl_repo"):
    if _p not in sys.path:
        sys.path.insert(0, _p)

import numpy as np
import ml_dtypes

import concourse.bass as bass  # noqa: F401  (registers types)
import concourse.mybir as mybir
import concourse.tile as tile
from concourse import bacc, bass_utils

F32 = mybir.dt.float32
F32R = mybir.dt.float32r
BF16 = mybir.dt.bfloat16

H = 4096          # hidden
S = 2048          # sequence
P = 128           # partitions
HD = 128          # head dim
NCORES = 8
HPC = 4           # heads per core
CW = HPC * HD     # per-core width of q/k/v/attn dims = 512
NCHUNKS = 4       # seq chunks of 512
KCH = H // P      # 32 hidden chunks
LORA_SCALING = 2.0
EXP_SCALE = float(1.0 / np.sqrt(HD))

_CACHE = {}


def _build():
    nc = bacc.Bacc("TRN2", target_bir_lowering=False, debug=False, num_devices=NCORES)

    # host-packed layouts (see kernel() for packing):
    #  xt:   [ncx, b, p, c*512+s]  (4, 4, 128, 4096)   8KB lines
    #  wq/wk/wv: [kb, p, ki*512+m] (8, 128, 2048)      4KB lines
    #  wot:  [p, h, n]             (128, 4, 4096)      resident, 8KB+ lines
    #  cosr/sinr: (128, 2048)      cos/sin duplicated across both halves
    xt_d = nc.declare_dram_parameter("xt", [NCHUNKS, 4, P, 8, 512], BF16, isOutput=False)
    wq_d = nc.declare_dram_parameter("wq", [KCH // 4, P, 4, CW], BF16, isOutput=False)
    wk_d = nc.declare_dram_parameter("wk", [KCH // 4, P, 4, CW], BF16, isOutput=False)
    wv_d = nc.declare_dram_parameter("wv", [KCH // 4, P, 4, CW], BF16, isOutput=False)
    wot_d = nc.declare_dram_parameter("wot", [P, HPC, H], BF16, isOutput=False)
    cosr_d = nc.declare_dram_parameter("cosr", [P, S], BF16, isOutput=False)
    sinr_d = nc.declare_dram_parameter("sinr", [P, S], BF16, isOutput=False)
    masks_d = nc.declare_dram_parameter("masks", [P, 4, 512], BF16, isOutput=False)
    ones_d = nc.declare_dram_parameter("ones", [P, P], BF16, isOutput=False)
    out_d = nc.declare_dram_parameter("out", [S, H], BF16, isOutput=True)

    with tile.TileContext(nc) as tc:
        with tc.tile_pool(name="persist", bufs=1) as pp:
            qt = [pp.tile([P, S], BF16, tag=f"qt{h}", name=f"qt{h}") for h in range(HPC)]
            kt = [pp.tile([P, S], BF16, tag=f"kt{h}", name=f"kt{h}") for h in range(HPC)]
            v_sb = pp.tile([P, S // P, CW], BF16, tag="v")   # (128, 16, 512)
            cos_sb = pp.tile([P, S], BF16, tag="cos")
            sin_sb = pp.tile([P, S], BF16, tag="sin")
            ones_sb = pp.tile([P, P], BF16, tag="ones")
            wot_sb = pp.tile([P, HPC, H], BF16, tag="wot")
            masks_sb = pp.tile([P, 4, 512], BF16, tag="masks")
            # consts on the scalar queue (idle at start)
            nc.scalar.dma_start(cos_sb[:], cosr_d[:])
            nc.scalar.dma_start(sin_sb[:], sinr_d[:])
            nc.scalar.dma_start(masks_sb[:], masks_d[:])
            nc.scalar.dma_start(ones_sb[:], ones_d[:])
            ones_col = ones_sb[:, 0:1]
            ones_row = ones_sb[0:1, :]

            def rope(qp, dest, ncx):
                """qp: (128,512) f32 PSUM Q^T/K^T chunk; dest: bf16 SBUF slice.
                dest[0:64] = q1*cos - q2*sin ; dest[64:] = q1*sin + q2*cos."""
                sl = slice(ncx * 512, (ncx + 1) * 512)
                c1 = ropep.tile([P, 512], BF16, tag="rc1", name="rc1")
                m1 = ropep.tile([P, 512], BF16, tag="rm1", name="rm1")
                m2 = ropep.tile([P, 512], BF16, tag="rm2", name="rm2")
                nc.scalar.copy(c1[:], qp[:])                      # psum -> sbuf bf16
                nc.vector.tensor_mul(m1[:], c1[:], cos_sb[:, sl])
                # sin_sb holds sin duplicated on both partition halves, so the
                # swapped-half muls can use partition-matched operands (the BIR
                # verifier requires equal base partitions for SBUF+SBUF pairs).
                nc.vector.tensor_mul(m2[0:64], c1[64:128], sin_sb[64:128, sl])
                nc.vector.tensor_mul(m2[64:128], c1[0:64], sin_sb[0:64, sl])
                nc.vector.tensor_sub(dest[0:64], m1[0:64], m2[0:64])
                nc.vector.tensor_add(dest[64:128], m1[64:128], m2[64:128])

            # ---------------- Phase 1: Q^T, K^T, V projections ----------------
            # Seq chunks processed in PAIRS sharing each weight DMA (halves
            # weight traffic to 24MB); each wave uses all 8 PSUM banks
            # (4 heads x 2 chunks).
            with tc.tile_pool(name="xtp", bufs=9) as xtp, \
                 tc.tile_pool(name="wp", bufs=3) as wp, \
                 tc.tile_pool(name="ropep", bufs=2) as ropep, \
                 tc.tile_pool(name="projps", bufs=8, space="PSUM") as projps:
                for npair in range(NCHUNKS // 2):
                    pair = (2 * npair, 2 * npair + 1)
                    xts = {}
                    for ncx in pair:
                        xts[ncx] = []
                        for b in range(4):
                            t = xtp.tile([P, 8, 512], BF16, tag="xt", name=f"xt{ncx}_{b}")
                            nc.gpsimd.dma_start(t[:], xt_d[ncx, b])
                            xts[ncx].append(t)

                    for wsel, w3 in (("q", wq_d), ("k", wk_d)):
                        ps = {ncx: [projps.tile([P, 512], F32, tag="proj",
                                                name=f"{wsel}_ps{ncx}_{i}")
                                    for i in range(HPC)] for ncx in pair}
                        for kb in range(KCH // 4):
                            w_t = wp.tile([P, 4, CW], BF16, tag="w", name=f"w{wsel}{npair}_{kb}")
                            nc.sync.dma_start(w_t[:], w3[kb])
                            for ki in range(4):
                                k = 4 * kb + ki
                                for m in range(HPC):
                                    lw = w_t[:, ki, m * HD:(m + 1) * HD]
                                    for ncx in pair:
                                        nc.tensor.matmul(
                                            ps[ncx][m][:], lhsT=lw,
                                            rhs=xts[ncx][k // 8][:, k % 8, :],
                                            start=(k == 0), stop=(k == KCH - 1))
                        dst = qt if wsel == "q" else kt
                        for ncx in pair:
                            for m in range(HPC):
                                rope(ps[ncx][m], dst[m][:, ncx * 512:(ncx + 1) * 512], ncx)

                    v_ps = {ncx: [projps.tile([P, 512], F32, tag="proj",
                                              name=f"v_ps{ncx}_{i}") for i in range(4)]
                            for ncx in pair}
                    for kb in range(KCH // 4):
                        w_t = wp.tile([P, 4, CW], BF16, tag="w", name=f"wv{npair}_{kb}")
                        nc.sync.dma_start(w_t[:], wv_d[kb])
                        for ki in range(4):
                            k = 4 * kb + ki
                            for ncx in pair:
                                for t in range(4):
                                    nc.tensor.matmul(
                                        v_ps[ncx][t][:],
                                        lhsT=xts[ncx][k // 8][:, k % 8, t * P:(t + 1) * P],
                                        rhs=w_t[:, ki, :], start=(k == 0), stop=(k == KCH - 1))
                    for ncx in pair:
                        for t in range(4):
                            nc.scalar.copy(v_sb[:, ncx * 4 + t, :], v_ps[ncx][t][:])

                # Wo load deferred to here (gpsimd queue) so it doesn't steal
                # HBM bandwidth from the startup-critical xt/w loads.
                nc.gpsimd.dma_start(wot_sb[:], wot_d[:])

            # ---------------- Phase 2: attention ----------------
            # Transposed scores S^T (k on partitions, q on free) in 2-k-tile
            # units; exp on scalar; probs bf16; softmax-normalization tail
            # (recip -> ones-broadcast matmul -> scale) runs one unit behind
            # so the tensor engine never waits on it.
            with tc.tile_pool(name="avtsp", bufs=18) as avtsp:
                avt_all = [[None] * HPC for _ in range(NCHUNKS)]

                with tc.tile_pool(name="probsp", bufs=5) as probsp, \
                     tc.tile_pool(name="paddp", bufs=3) as paddp, \
                     tc.tile_pool(name="recp", bufs=2) as recp, \
                     tc.tile_pool(name="rbp", bufs=2) as rbp, \
                     tc.tile_pool(name="stps", bufs=2, space="PSUM") as stps, \
                     tc.tile_pool(name="avtps", bufs=2, space="PSUM") as avtps, \
                     tc.tile_pool(name="smallps", bufs=2, space="PSUM") as smallps:

                    def tail(prev):
                        qc, h, avt_ps, sums_ps = prev
                        recip_f = recp.tile([1, 512], F32, tag="recf", name="recf")
                        scratch = recp.tile([1, 512], F32, tag="recs", name="recs")
                        recip_b = recp.tile([1, 512], BF16, tag="recb", name="recb")
                        nc.vector.reciprocal_approx_accurate(
                            out=recip_f[:], in_=sums_ps[:], scratch=scratch[:])
                        nc.vector.tensor_copy(recip_b[:], recip_f[:])
                        rb_ps = smallps.tile([P, 512], F32, tag="small", name="rb_ps")
                        nc.tensor.matmul(rb_ps[:], lhsT=ones_row, rhs=recip_b[:],
                                         start=True, stop=True)
                        rb_sb = rbp.tile([P, 512], BF16, tag="rb", name="rb_sb")
                        nc.vector.tensor_copy(rb_sb[:], rb_ps[:])
                        avs = avtsp.tile([P, 512], BF16, tag="avts",
                                         name=f"avts{qc}_{h}")
                        nc.vector.tensor_mul(avs[:], avt_ps[:], rb_sb[:])
                        avt_all[qc][h] = avs

                    # Score matmuls are emitted one 2-k-tile unit AHEAD of
                    # their consumers (exp/mask/AV/sums) so the tensor queue
                    # always holds a runnable matmul while the exp chain of
                    # the previous unit drains.
                    from collections import deque

                    def emit_scores(qc, h, ktb):
                        st2 = stps.tile([P, 2, 512], F32, tag="st", name="st2")
                        qsl = slice(qc * 512, (qc + 1) * 512)
                        for u in range(2):
                            kti = 2 * ktb + u
                            nc.tensor.matmul(
                                st2[:, u, :], lhsT=kt[h][:, kti * P:(kti + 1) * P],
                                rhs=qt[h][:, qsl], start=True, stop=True)
                        return st2

                    state = {}

                    def emit_consume(qc, h, ktb, st2, avt_ps, sums_ps):
                        nkt = 4 * (qc + 1)
                        probs2 = probsp.tile([P, 2, 512], BF16, tag="probs", name="probs2")
                        nc.scalar.activation(probs2[:], st2[:],
                                             mybir.ActivationFunctionType.Exp,
                                             scale=EXP_SCALE)
                        j = 2 * ktb - 4 * qc
                        if j >= 0:
                            nc.vector.tensor_mul(probs2[:], probs2[:],
                                                 masks_sb[:, j:j + 2, :])
                        padd = paddp.tile([P, 512], BF16, tag="padd", name="padd")
                        nc.vector.tensor_add(padd[:], probs2[:, 0, :], probs2[:, 1, :])
                        for u in range(2):
                            kti = 2 * ktb + u
                            nc.tensor.matmul(
                                avt_ps[:], lhsT=v_sb[:, kti, h * HD:(h + 1) * HD],
                                rhs=probs2[:, u, :],
                                start=(kti == 0), stop=(kti == nkt - 1))
                        if ktb % 2 == 0:
                            state["padd_prev"] = padd
                        else:
                            padd2 = paddp.tile([P, 512], BF16, tag="padd2", name="padd2")
                            nc.vector.tensor_add(padd2[:], state["padd_prev"][:], padd[:])
                            nc.tensor.matmul(
                                sums_ps[:], lhsT=ones_col, rhs=padd2[:],
                                start=(ktb == 1), stop=(ktb == nkt // 2 - 1))

                    prev = None
                    pending = deque()
                    for qc in range(NCHUNKS):
                        for h in range(HPC):
                            avt_ps = avtps.tile([P, 512], F32, tag="avt", name="avt_ps")
                            sums_ps = smallps.tile([1, 512], F32, tag="small", name="sums_ps")
                            nkt = 4 * (qc + 1)
                            for ktb in range(nkt // 2):
                                st2 = emit_scores(qc, h, ktb)
                                pending.append((qc, h, ktb, st2, avt_ps, sums_ps))
                                if len(pending) > 1:
                                    item = pending.popleft()
                                    emit_consume(*item)
                                    if item[2] == 0 and prev is not None:
                                        tail(prev)
                                        prev = None
                            prev_unit = (qc, h, avt_ps, sums_ps)
                            if qc == NCHUNKS - 1 and h == HPC - 1:
                                # flush: last unit's pending consumes + tails
                                while pending:
                                    item = pending.popleft()
                                    emit_consume(*item)
                                    if item[2] == 0 and prev is not None:
                                        tail(prev)
                                        prev = None
                            prev = prev_unit
                    tail(prev)

                # ---------------- Phase 3: out-proj ----------------
                # Wo resident; hc processed in pairs so the bf16 output DMA
                # writes 2KB contiguous lines. PSUM evacuation alternates
                # scalar/vector engines.
                with tc.tile_pool(name="osbp", bufs=4) as osbp, \
                     tc.tile_pool(name="outps", bufs=8, space="PSUM") as outps:
                    cp = 0
                    for hcp in range(4):
                        for qc in range(NCHUNKS):
                            for qs in range(4):
                                o_sb = osbp.tile([P, 1024], BF16, tag="osb", name="o_sb")
                                o_ps = [outps.tile([P, 512], F32, tag="o", name="o_ps")
                                        for _ in range(2)]
                                # h outer / hc-pair inner: consecutive matmuls
                                # share the stationary avs chunk.
                                for h in range(HPC):
                                    lw = avt_all[qc][h][:, qs * P:(qs + 1) * P]
                                    for hsub in range(2):
                                        hc = 2 * hcp + hsub
                                        nc.tensor.matmul(
                                            o_ps[hsub][:], lhsT=lw,
                                            rhs=wot_sb[:, h, hc * 512:(hc + 1) * 512],
                                            start=(h == 0), stop=(h == HPC - 1))
                                for hsub in range(2):
                                    if cp % 2 == 0:
                                        nc.scalar.copy(
                                            o_sb[:, hsub * 512:(hsub + 1) * 512], o_ps[hsub][:])
                                    else:
                                        nc.vector.tensor_copy(
                                            o_sb[:, hsub * 512:(hsub + 1) * 512], o_ps[hsub][:])
                                    cp += 1
                                nc.sync.dma_start(
                                    out_d[qc * 512 + qs * P: qc * 512 + (qs + 1) * P,
                                          hcp * 1024:(hcp + 1) * 1024],
                                    o_sb[:])

    nc.compile()
    return nc


def _fold(W, A, B):
    """Fold LoRA + its half/interleave permutation into the base weight."""
    BA = (B.astype(np.float64) @ A.astype(np.float64)) * LORA_SCALING
    j = np.arange(H)
    g = np.where(j < H // 2, 2 * j, 2 * (j - H // 2) + 1)
    return (W.astype(np.float64) + BA[g, :]).astype(np.float32)


def _host_consts():
    inv_freq = (1.0 / (10000.0 ** (np.arange(0, HD, 2, dtype=np.float32) / HD))).astype(np.float32)
    freqs = np.arange(S, dtype=np.float32)[:, None] * inv_freq[None, :]   # (S, 64)
    cosr = np.concatenate([np.cos(freqs).T] * 2, axis=0)   # (128, S), both halves cos
    sinr = np.concatenate([np.sin(freqs).T] * 2, axis=0)
    p = np.arange(P)[:, None, None]
    jj = np.arange(4)[None, :, None]
    f = np.arange(512)[None, None, :]
    masks = (jj * P + p <= f).astype(np.float32)          # (128, 4, 512)
    ones = np.ones((P, P), dtype=np.float32)
    bf = ml_dtypes.bfloat16
    return cosr.astype(bf), sinr.astype(bf), masks.astype(bf), ones.astype(bf)


def _pack_xt(x):
    """x: (S, H) f32 -> (4, 4, 128, 4096) bf16 with [ncx, b, p, c*512+s]."""
    XT = np.ascontiguousarray(x.T)                         # (H, S)
    a = XT.reshape(4, 8, P, NCHUNKS, 512)                  # (b, c, p, ncx, s)
    a = a.transpose(3, 0, 2, 1, 4)                         # (ncx, b, p, c, s)
    return np.ascontiguousarray(a).astype(ml_dtypes.bfloat16)


def _pack_w(Wcols):
    """Wcols: (H, CW) f32 (= W_eff[cols].T) -> (8, 128, 2048) bf16 [kb, p, ki*512+m]."""
    a = Wcols.reshape(KCH // 4, 4, P, CW).transpose(0, 2, 1, 3)   # (kb, p, ki, m)
    return np.ascontiguousarray(a).astype(ml_dtypes.bfloat16)


def _pack_wot(WoT):
    """WoT: (CW, H) f32 (= Wo[:, cols].T) -> (128, 4, 4096) bf16 [p, h, n]."""
    a = WoT.reshape(HPC, P, H).transpose(1, 0, 2)
    return np.ascontiguousarray(a).astype(ml_dtypes.bfloat16)


def kernel(hidden_states, Wq, Wk, Wv, Wo, Aq, Bq, Ak, Bk, Av, Bv):
    if "nc" not in _CACHE:
        _CACHE["nc"] = _build()
    nc = _CACHE["nc"]

    x = np.ascontiguousarray(np.asarray(hidden_states, dtype=np.float32)[0])  # (S, H)
    xt_p = _pack_xt(x)

    Wq_eff = _fold(np.asarray(Wq), np.asarray(Aq), np.asarray(Bq))
    Wk_eff = _fold(np.asarray(Wk), np.asarray(Ak), np.asarray(Bk))
    Wv_eff = _fold(np.asarray(Wv), np.asarray(Av), np.asarray(Bv))
    Wo_np = np.asarray(Wo, dtype=np.float32)

    cosr, sinr, masks, ones = _host_consts()

    in_maps = []
    for c in range(NCORES):
        cols = slice(CW * c, CW * (c + 1))
        in_maps.append({
            "xt": xt_p,
            "wq": _pack_w(np.ascontiguousarray(Wq_eff[cols].T)),
            "wk": _pack_w(np.ascontiguousarray(Wk_eff[cols].T)),
            "wv": _pack_w(np.ascontiguousarray(Wv_eff[cols].T)),
            "wot": _pack_wot(np.ascontiguousarray(Wo_np[:, cols].T)),
            "cosr": cosr,
            "sinr": sinr,
            "masks": masks,
            "ones": ones,
        })
    _CACHE["in_maps"] = in_maps

    res = bass_utils.run_bass_kernel_spmd(nc, in_maps, core_ids=list(range(NCORES)))
    acc = np.zeros((S, H), dtype=np.float64)
    for c in range(NCORES):
        acc += res.results[c]["out"].astype(np.float64)
    return acc.astype(np.float32)[None]


# revision 16
# speedup vs baseline: 1.0314x; 1.0314x over previous
"""Trainium2 Bass kernel for ConvertedLlamaAttention (LoRA q/k/v + RoPE + causal attention + out-proj).

Strategy: tensor-parallel over heads across 8 NeuronCores (4 heads/core).
All device matmuls run in "transposed" layouts so no on-device transposes are
needed anywhere:
  - Q^T, K^T computed as W^T-stationary matmuls (head_dim on partitions),
  - V computed in natural layout (seq on partitions) from the same X^T tiles,
  - scores computed transposed (S^T = K^T^T-slices @ Q^T) so softmax sums are
    done with a ones-vector matmul, and A·V consumes V in natural layout,
  - out-proj consumes A·V^T directly as the stationary operand.
LoRA (incl. the half-interleave) is folded into the weights on the host.
Each core emits a partial (2048, 4096) bf16 output (row-parallel Wo); host sums.

v4:
  - host-packed DRAM layouts for wide contiguous DMA lines; startup-critical
    xt/w tiles split into sub-DMAs (subtile deps let matmuls start early);
    consts deferred behind first-wave weights on the sync queue,
  - rope: scalar-engine PSUM evacuation to bf16, DVE muls in bf16 (4x mode),
  - bf16 everywhere off the f32 PSUM accumulators,
  - phase 2: scores emitted one unit ahead of consumers; diagonal blocks
    processed as per-128-tile jobs with the masked upper-triangle region
    trimmed from scores/exp/AV/sums; softmax-normalization tail software-
    pipelined one unit behind,
  - Wo resident in SBUF; bf16 output with 2KB lines; PSUM evacuation
    alternates scalar/vector engines.
"""
import sys

for _p in ("/opt/trn_rl_repo", "/root/.axon_site/_ro/trn_rl_repo"):
    if _p not in sys.path:
        sys.path.insert(0, _p)

from collections import deque

import numpy as np
import ml_dtypes

import concourse.bass as bass  # noqa: F401  (registers types)
import concourse.mybir as mybir
import concourse.tile as tile
from concourse import bacc, bass_utils

F32 = mybir.dt.float32
BF16 = mybir.dt.bfloat16

H = 4096          # hidden
S = 2048          # sequence
P = 128           # partitions
HD = 128          # head dim
NCORES = 8
HPC = 4           # heads per core
CW = HPC * HD     # per-core width of q/k/v/attn dims = 512
NCHUNKS = 4       # seq chunks of 512
KCH = H // P      # 32 hidden chunks
LORA_SCALING = 2.0
EXP_SCALE = float(1.0 / np.sqrt(HD))

_CACHE = {}


def _build():
    nc = bacc.Bacc("TRN2", target_bir_lowering=False, debug=False, num_devices=NCORES)

    # host-packed layouts (see kernel() for packing):
    #  xt:   [ncx, b, p, c, s]   (4, 4, 128, 8, 512)   8KB lines
    #  wq/wk/wv: [kb, p, ki, m]  (8, 128, 4, 512)      4KB lines
    #  wot:  [p, h, n]           (128, 4, 4096)        resident
    #  cosr/sinr: (128, 2048)    cos/sin duplicated across both halves
    xt_d = nc.declare_dram_parameter("xt", [NCHUNKS, 4, P, 8, 512], BF16, isOutput=False)
    wq_d = nc.declare_dram_parameter("wq", [KCH // 4, P, 4, CW], BF16, isOutput=False)
    wk_d = nc.declare_dram_parameter("wk", [KCH // 4, P, 4, CW], BF16, isOutput=False)
    wv_d = nc.declare_dram_parameter("wv", [KCH // 4, P, 4, CW], BF16, isOutput=False)
    wot_d = nc.declare_dram_parameter("wot", [P, HPC, H], BF16, isOutput=False)
    cosr_d = nc.declare_dram_parameter("cosr", [P, S], BF16, isOutput=False)
    sinr_d = nc.declare_dram_parameter("sinr", [P, S], BF16, isOutput=False)
    masks_d = nc.declare_dram_parameter("masks", [P, 4, 512], BF16, isOutput=False)
    ones_d = nc.declare_dram_parameter("ones", [P, P], BF16, isOutput=False)
    out_d = nc.declare_dram_parameter("out", [S, H], BF16, isOutput=True)

    with tile.TileContext(nc) as tc:
        with tc.tile_pool(name="persist", bufs=1) as pp:
            qt = [pp.tile([P, S], BF16, tag=f"qt{h}", name=f"qt{h}") for h in range(HPC)]
            kt = [pp.tile([P, S], BF16, tag=f"kt{h}", name=f"kt{h}") for h in range(HPC)]
            v_sb = pp.tile([P, S // P, CW], BF16, tag="v")   # (128, 16, 512)
            cos_sb = pp.tile([P, S], BF16, tag="cos")
            sin_sb = pp.tile([P, S], BF16, tag="sin")
            ones_sb = pp.tile([P, P], BF16, tag="ones")
            wot_sb = pp.tile([P, HPC, H], BF16, tag="wot")
            masks_sb = pp.tile([P, 4, 512], BF16, tag="masks")
            ones_col = ones_sb[:, 0:1]
            ones_row = ones_sb[0:1, :]

            def rope(qp, dest, ncx):
                """qp: (128,512) f32 PSUM Q^T/K^T chunk; dest: bf16 SBUF slice.
                dest[0:64] = q1*cos - q2*sin ; dest[64:] = q1*sin + q2*cos."""
                sl = slice(ncx * 512, (ncx + 1) * 512)
                c1 = ropep.tile([P, 512], BF16, tag="rc1", name="rc1")
                m1 = ropep.tile([P, 512], BF16, tag="rm1", name="rm1")
                m2 = ropep.tile([P, 512], BF16, tag="rm2", name="rm2")
                nc.scalar.copy(c1[:], qp[:])                      # psum -> sbuf bf16
                nc.vector.tensor_mul(m1[:], c1[:], cos_sb[:, sl])
                # sin_sb holds sin duplicated on both partition halves, so the
                # swapped-half muls use partition-matched SBUF operands (BIR
                # requires equal base partitions for SBUF+SBUF pairs).
                nc.vector.tensor_mul(m2[0:64], c1[64:128], sin_sb[64:128, sl])
                nc.vector.tensor_mul(m2[64:128], c1[0:64], sin_sb[0:64, sl])
                nc.vector.tensor_sub(dest[0:64], m1[0:64], m2[0:64])
                nc.vector.tensor_add(dest[64:128], m1[64:128], m2[64:128])

            # ---------------- Phase 1: Q^T, K^T, V projections ----------------
            # 4-PSUM waves, double-buffered via projps bufs=8 so consecutive
            # waves pipeline while rope drains the finished wave's banks.
            consts_emitted = False
            with tc.tile_pool(name="xtp", bufs=6) as xtp, \
                 tc.tile_pool(name="wp", bufs=3) as wp, \
                 tc.tile_pool(name="ropep", bufs=2) as ropep, \
                 tc.tile_pool(name="projps", bufs=8, space="PSUM") as projps:
                for ncx in range(NCHUNKS):
                    xts = []
                    for b in range(4):
                        t = xtp.tile([P, 8, 512], BF16, tag="xt", name=f"xt{ncx}_{b}")
                        if ncx == 0 and b == 0:
                            # split the startup-critical first tile so the
                            # first matmuls only wait on a 256KB sub-DMA
                            for c2 in range(4):
                                nc.gpsimd.dma_start(t[:, 2 * c2:2 * c2 + 2, :],
                                                    xt_d[ncx, b, :, 2 * c2:2 * c2 + 2, :])
                        else:
                            nc.gpsimd.dma_start(t[:], xt_d[ncx, b])
                        xts.append(t)

                    for wsel, w3 in (("q", wq_d), ("k", wk_d)):
                        ps = [projps.tile([P, 512], F32, tag="proj",
                                          name=f"{wsel}_ps{ncx}_{i}") for i in range(HPC)]
                        for kb in range(KCH // 4):
                            w_t = wp.tile([P, 4, CW], BF16, tag="w", name=f"w{wsel}{ncx}_{kb}")
                            if ncx == 0 and wsel == "q" and kb == 0:
                                for ki4 in range(4):
                                    nc.sync.dma_start(w_t[:, ki4, :], w3[kb, :, ki4, :])
                            else:
                                nc.sync.dma_start(w_t[:], w3[kb])
                            for ki in range(4):
                                k = 4 * kb + ki
                                rhs = xts[k // 8][:, k % 8, :]
                                for m in range(HPC):
                                    nc.tensor.matmul(
                                        ps[m][:], lhsT=w_t[:, ki, m * HD:(m + 1) * HD],
                                        rhs=rhs, start=(k == 0), stop=(k == KCH - 1))
                        if not consts_emitted:
                            # consts ride the sync queue AFTER the first
                            # wave's weights so they don't steal startup BW
                            nc.sync.dma_start(cos_sb[:], cosr_d[:])
                            nc.sync.dma_start(sin_sb[:], sinr_d[:])
                            nc.sync.dma_start(masks_sb[:], masks_d[:])
                            nc.sync.dma_start(ones_sb[:], ones_d[:])
                            consts_emitted = True
                        dst = qt if wsel == "q" else kt
                        for m in range(HPC):
                            rope(ps[m], dst[m][:, ncx * 512:(ncx + 1) * 512], ncx)

                    v_ps = [projps.tile([P, 512], F32, tag="proj",
                                        name=f"v_ps{ncx}_{i}") for i in range(4)]
                    for kb in range(KCH // 4):
                        w_t = wp.tile([P, 4, CW], BF16, tag="w", name=f"wv{ncx}_{kb}")
                        nc.sync.dma_start(w_t[:], wv_d[kb])
                        for ki in range(4):
                            k = 4 * kb + ki
                            for t in range(4):
                                nc.tensor.matmul(
                                    v_ps[t][:],
                                    lhsT=xts[k // 8][:, k % 8, t * P:(t + 1) * P],
                                    rhs=w_t[:, ki, :], start=(k == 0), stop=(k == KCH - 1))
                    for t in range(4):
                        nc.scalar.copy(v_sb[:, ncx * 4 + t, :], v_ps[t][:])

                # Wo load deferred to here (gpsimd queue) so it doesn't steal
                # HBM bandwidth from the startup-critical xt/w loads.
                nc.gpsimd.dma_start(wot_sb[:], wot_d[:])

            # ---------------- Phase 2: attention ----------------
            # Transposed scores S^T (k on partitions, q on free). Fully-causal
            # k-tiles run as 2-wide units; the 4 diagonal k-tiles run as
            # single jobs with the masked q-range trimmed away entirely
            # (scores/exp/AV/sums all skip it). Scores are emitted one job
            # ahead of consumers; the normalization tail (recip -> ones-
            # broadcast matmul -> scale) runs one unit behind.
            with tc.tile_pool(name="avtsp", bufs=18) as avtsp:
                avt_all = [[None] * HPC for _ in range(NCHUNKS)]

                with tc.tile_pool(name="probsp", bufs=6) as probsp, \
                     tc.tile_pool(name="paddp", bufs=3) as paddp, \
                     tc.tile_pool(name="recp", bufs=2) as recp, \
                     tc.tile_pool(name="rbp", bufs=2) as rbp, \
                     tc.tile_pool(name="stps", bufs=2, space="PSUM") as stps, \
                     tc.tile_pool(name="avtps", bufs=2, space="PSUM") as avtps, \
                     tc.tile_pool(name="smallps", bufs=2, space="PSUM") as smallps:

                    def tail(prev):
                        qc, h, avt_ps, sums_ps = prev
                        recip_f = recp.tile([1, 512], F32, tag="recf", name="recf")
                        scratch = recp.tile([1, 512], F32, tag="recs", name="recs")
                        recip_b = recp.tile([1, 512], BF16, tag="recb", name="recb")
                        nc.vector.reciprocal_approx_accurate(
                            out=recip_f[:], in_=sums_ps[:], scratch=scratch[:])
                        nc.vector.tensor_copy(recip_b[:], recip_f[:])
                        rb_ps = smallps.tile([P, 512], F32, tag="small", name="rb_ps")
                        nc.tensor.matmul(rb_ps[:], lhsT=ones_row, rhs=recip_b[:],
                                         start=True, stop=True)
                        rb_sb = rbp.tile([P, 512], BF16, tag="rb", name="rb_sb")
                        nc.vector.tensor_copy(rb_sb[:], rb_ps[:])
                        avs = avtsp.tile([P, 512], BF16, tag="avts",
                                         name=f"avts{qc}_{h}")
                        nc.vector.tensor_mul(avs[:], avt_ps[:], rb_sb[:])
                        avt_all[qc][h] = avs

                    def emit_scores(ctx, job):
                        qc, h = ctx["qc"], ctx["h"]
                        st = stps.tile([P, 2, 512], F32, tag="st", name="st")
                        kind, a = job
                        if kind == "pair":
                            for u in range(2):
                                kti = 2 * a + u
                                nc.tensor.matmul(
                                    st[:, u, :], lhsT=kt[h][:, kti * P:(kti + 1) * P],
                                    rhs=qt[h][:, qc * 512:(qc + 1) * 512],
                                    start=True, stop=True)
                        else:
                            l = a
                            kti = 4 * qc + l
                            w = 512 - l * P
                            nc.tensor.matmul(
                                st[:, 0, 0:w], lhsT=kt[h][:, kti * P:(kti + 1) * P],
                                rhs=qt[h][:, qc * 512 + l * P:(qc + 1) * 512],
                                start=True, stop=True)
                        return st

                    def emit_consume(ctx, job, st):
                        qc, h = ctx["qc"], ctx["h"]
                        nkt = 4 * (qc + 1)
                        kind, a = job
                        if kind == "pair":
                            ktb = a
                            probs2 = probsp.tile([P, 2, 512], BF16, tag="probs",
                                                 name="probs2")
                            nc.scalar.activation(probs2[:], st[:],
                                                 mybir.ActivationFunctionType.Exp,
                                                 scale=EXP_SCALE)
                            for u in range(2):
                                kti = 2 * ktb + u
                                nc.tensor.matmul(
                                    ctx["avt"][:], lhsT=v_sb[:, kti, h * HD:(h + 1) * HD],
                                    rhs=probs2[:, u, :],
                                    start=(kti == 0), stop=False)
                            if ktb % 2 == 0:
                                ctx["padd_prev"] = probs2
                            else:
                                pprev = ctx["padd_prev"]
                                padd = paddp.tile([P, 512], BF16, tag="padd", name="padd")
                                padd2 = paddp.tile([P, 512], BF16, tag="padd2", name="padd2")
                                nc.vector.tensor_add(padd[:], pprev[:, 0, :], pprev[:, 1, :])
                                nc.vector.tensor_add(padd2[:], probs2[:, 0, :], probs2[:, 1, :])
                                nc.vector.tensor_add(padd2[:], padd[:], padd2[:])
                                nc.tensor.matmul(
                                    ctx["sums"][:], lhsT=ones_col, rhs=padd2[:],
                                    start=(ctx["sums_n"] == 0), stop=False)
                                ctx["sums_n"] += 1
                        else:
                            l = a
                            kti = 4 * qc + l
                            w = 512 - l * P
                            probs = probsp.tile([P, 2, 512], BF16, tag="probs",
                                                name="probs1")
                            nc.scalar.activation(probs[:, 0, 0:w], st[:, 0, 0:w],
                                                 mybir.ActivationFunctionType.Exp,
                                                 scale=EXP_SCALE)
                            # in-tile causal triangle: p <= f_local on the
                            # first 128 columns of this job's q-range
                            nc.vector.tensor_mul(probs[:, 0, 0:P], probs[:, 0, 0:P],
                                                 masks_sb[:, 0, 0:P])
                            nc.tensor.matmul(
                                ctx["avt"][:, l * P:512],
                                lhsT=v_sb[:, kti, h * HD:(h + 1) * HD],
                                rhs=probs[:, 0, 0:w],
                                start=(kti == 0), stop=(kti == nkt - 1))
                            if l == 0:
                                ctx["diag_acc"] = probs
                            else:
                                acc = ctx["diag_acc"]
                                nc.vector.tensor_add(acc[:, 0, l * P:512],
                                                     acc[:, 0, l * P:512],
                                                     probs[:, 0, 0:w])
                            if l == 3:
                                nc.tensor.matmul(
                                    ctx["sums"][:], lhsT=ones_col,
                                    rhs=ctx["diag_acc"][:, 0, :],
                                    start=(ctx["sums_n"] == 0), stop=True)
                                ctx["sums_n"] += 1

                    prev = None
                    pending = deque()
                    units = [(qc, h) for qc in range(NCHUNKS) for h in range(HPC)]
                    for qc, h in units:
                        ctx = {
                            "qc": qc, "h": h,
                            "avt": avtps.tile([P, 512], F32, tag="avt", name="avt_ps"),
                            "sums": smallps.tile([1, 512], F32, tag="small",
                                                 name="sums_ps"),
                            "sums_n": 0,
                        }
                        jobs = [("pair", ktb) for ktb in range(2 * qc)] + \
                               [("single", l) for l in range(4)]
                        for ji, job in enumerate(jobs):
                            st = emit_scores(ctx, job)
                            pending.append((ctx, job, ji, st))
                            if len(pending) > 1:
                                c, j, i, s = pending.popleft()
                                emit_consume(c, j, s)
                                if i == 0 and prev is not None:
                                    tail(prev)
                                    prev = None
                        prev_unit = (qc, h, ctx["avt"], ctx["sums"])
                        if (qc, h) == units[-1]:
                            while pending:
                                c, j, i, s = pending.popleft()
                                emit_consume(c, j, s)
                                if i == 0 and prev is not None:
                                    tail(prev)
                                    prev = None
                        prev = prev_unit
                    tail(prev)

                # ---------------- Phase 3: out-proj ----------------
                # Wo resident; hc processed in pairs so the bf16 output DMA
                # writes 2KB contiguous lines; h outer / hc-pair inner so
                # consecutive matmuls share the stationary avs chunk. PSUM
                # evacuation alternates scalar/vector engines.
                with tc.tile_pool(name="osbp", bufs=4) as osbp, \
                     tc.tile_pool(name="outps", bufs=8, space="PSUM") as outps:
                    cp = 0
                    for hcp in range(4):
                        for qc in range(NCHUNKS):
                            for qs in range(4):
                                o_sb = osbp.tile([P, 1024], BF16, tag="osb", name="o_sb")
                                o_ps = [outps.tile([P, 512], F32, tag="o", name="o_ps")
                                        for _ in range(2)]
                                for h in range(HPC):
                                    lw = avt_all[qc][h][:, qs * P:(qs + 1) * P]
                                    for hsub in range(2):
                                        hc = 2 * hcp + hsub
                                        nc.tensor.matmul(
                                            o_ps[hsub][:], lhsT=lw,
                                            rhs=wot_sb[:, h, hc * 512:(hc + 1) * 512],
                                            start=(h == 0), stop=(h == HPC - 1))
                                for hsub in range(2):
                                    if cp % 2 == 0:
                                        nc.scalar.copy(
                                            o_sb[:, hsub * 512:(hsub + 1) * 512],
                                            o_ps[hsub][:])
                                    else:
                                        nc.vector.tensor_copy(
                                            o_sb[:, hsub * 512:(hsub + 1) * 512],
                                            o_ps[hsub][:])
                                    cp += 1
                                nc.sync.dma_start(
                                    out_d[qc * 512 + qs * P: qc * 512 + (qs + 1) * P,
                                          hcp * 1024:(hcp + 1) * 1024],
                                    o_sb[:])

    nc.compile()
    return nc


def _fold(W, A, B):
    """Fold LoRA + its half/interleave permutation into the base weight."""
    BA = (B.astype(np.float64) @ A.astype(np.float64)) * LORA_SCALING
    j = np.arange(H)
    g = np.where(j < H // 2, 2 * j, 2 * (j - H // 2) + 1)
    return (W.astype(np.float64) + BA[g, :]).astype(np.float32)


def _host_consts():
    inv_freq = (1.0 / (10000.0 ** (np.arange(0, HD, 2, dtype=np.float32) / HD))).astype(np.float32)
    freqs = np.arange(S, dtype=np.float32)[:, None] * inv_freq[None, :]   # (S, 64)
    cosr = np.concatenate([np.cos(freqs).T] * 2, axis=0)   # (128, S), both halves cos
    sinr = np.concatenate([np.sin(freqs).T] * 2, axis=0)
    p = np.arange(P)[:, None, None]
    jj = np.arange(4)[None, :, None]
    f = np.arange(512)[None, None, :]
    masks = (jj * P + p <= f).astype(np.float32)          # (128, 4, 512)
    ones = np.ones((P, P), dtype=np.float32)
    bf = ml_dtypes.bfloat16
    return cosr.astype(bf), sinr.astype(bf), masks.astype(bf), ones.astype(bf)


def _pack_xt(x):
    """x: (S, H) f32 -> (4, 4, 128, 8, 512) bf16 with [ncx, b, p, c, s]."""
    XT = np.ascontiguousarray(x.T)                         # (H, S)
    a = XT.reshape(4, 8, P, NCHUNKS, 512)                  # (b, c, p, ncx, s)
    a = a.transpose(3, 0, 2, 1, 4)                         # (ncx, b, p, c, s)
    return np.ascontiguousarray(a).astype(ml_dtypes.bfloat16)


def _pack_w(Wcols):
    """Wcols: (H, CW) f32 (= W_eff[cols].T) -> (8, 128, 4, 512) bf16 [kb, p, ki, m]."""
    a = Wcols.reshape(KCH // 4, 4, P, CW).transpose(0, 2, 1, 3)   # (kb, p, ki, m)
    return np.ascontiguousarray(a).astype(ml_dtypes.bfloat16)


def _pack_wot(WoT):
    """WoT: (CW, H) f32 (= Wo[:, cols].T) -> (128, 4, 4096) bf16 [p, h, n]."""
    a = WoT.reshape(HPC, P, H).transpose(1, 0, 2)
    return np.ascontiguousarray(a).astype(ml_dtypes.bfloat16)


def kernel(hidden_states, Wq, Wk, Wv, Wo, Aq, Bq, Ak, Bk, Av, Bv):
    if "nc" not in _CACHE:
        _CACHE["nc"] = _build()
    nc = _CACHE["nc"]

    x = np.ascontiguousarray(np.asarray(hidden_states, dtype=np.float32)[0])  # (S, H)
    xt_p = _pack_xt(x)

    Wq_eff = _fold(np.asarray(Wq), np.asarray(Aq), np.asarray(Bq))
    Wk_eff = _fold(np.asarray(Wk), np.asarray(Ak), np.asarray(Bk))
    Wv_eff = _fold(np.asarray(Wv), np.asarray(Av), np.asarray(Bv))
    Wo_np = np.asarray(Wo, dtype=np.float32)

    cosr, sinr, masks, ones = _host_consts()

    in_maps = []
    for c in range(NCORES):
        cols = slice(CW * c, CW * (c + 1))
        in_maps.append({
            "xt": xt_p,
            "wq": _pack_w(np.ascontiguousarray(Wq_eff[cols].T)),
            "wk": _pack_w(np.ascontiguousarray(Wk_eff[cols].T)),
            "wv": _pack_w(np.ascontiguousarray(Wv_eff[cols].T)),
            "wot": _pack_wot(np.ascontiguousarray(Wo_np[:, cols].T)),
            "cosr": cosr,
            "sinr": sinr,
            "masks": masks,
            "ones": ones,
        })
    _CACHE["in_maps"] = in_maps

    res = bass_utils.run_bass_kernel_spmd(nc, in_maps, core_ids=list(range(NCORES)))
    acc = np.zeros((S, H), dtype=np.float64)
    for c in range(NCORES):
        acc += res.results[c]["out"].astype(np.float64)
    return acc.astype(np.float32)[None]
